# revision 1
# baseline (speedup 1.0000x reference)
"""
DenseEquivariantIrrep kernel for 8x Trainium2 NeuronCores.

Math: the reference computes, per batch row b:
    y[b, f, s] = sum_{c,t} x[b, c, t] * W[c, t, f, s] + bias[f]
where W folds the group-Fourier transform (fwd), the per-irrep block
matmul with the kernel, and the inverse transform (inv).  W depends only
on (kernel, irreps) which are tiny, so it is folded on the host; the
device does the batch-scaled work: a [4096, 1536] x [1536, 1536] matmul
chain per core (8-way batch-parallel, no cross-core communication).

Device pipeline per 512-row super-block:
    DMA x natural -> PE-transpose to x^T tiles [128(ct), 512(b)]
    -> 12x12 accumulating matmuls (y^T = W^T x^T) in PSUM
    -> bias-add fused into PSUM->SBUF copyback
    -> PE-transpose back to natural y -> DMA out.
"""

import numpy as np

import concourse.bass as bass
import concourse.mybir as mybir
from concourse import bacc
from concourse.tile import TileContext
from concourse.bass_utils import run_bass_kernel_spmd

N_CORES = 8
B, C, F, NS = 32768, 32, 32, 48
CT = C * NS   # 1536 contraction size
FS = F * NS   # 1536 output features
BS = B // N_CORES  # 4096 rows per core
SUPER = 512        # b-rows per super-block
KT = CT // 128     # 12 K tiles
MT = FS // 128     # 12 M (output) tiles

# dtype knob for matmul/transpose operands:
#   mybir.dt.float32r : PE full-rate (1 cyc/col at N>=256), numerics TBD on HW
#   mybir.dt.float32  : exact, 4 cyc/col
#   mybir.dt.bfloat16 : 1 cyc/col, ~4e-3 rounding
MM_DT = mybir.dt.float32r  # transposes / identity (exact pass-through)
W_DT = mybir.dt.float16    # middle matmul operands (FWL-eligible, ~5e-4 rounding)


def _host_fold(kernel, bias, irreps_d1, irreps_d2, irreps_d3):
    """Fold fwd/inv Fourier matrices and kernel into W[(c,t),(f,s)] + bias[(f,s)]."""
    groups = [np.asarray(irreps_d1), np.asarray(irreps_d2), np.asarray(irreps_d3)]
    n = NS
    fwd = np.concatenate(
        [g.transpose(1, 0, 2, 3).reshape(n, -1) for g in groups], axis=1
    ).astype(np.float64)
    inv = np.concatenate(
        [g.transpose(1, 0, 2, 3).reshape(n, -1) * (g.shape[-1] / n) for g in groups],
        axis=1,
    ).T.astype(np.float64)
    kh = np.asarray(kernel).astype(np.float64) @ fwd  # [F, C, 48]
    W = np.zeros((C, NS, F, NS), np.float64)
    off = 0
    for g in groups:
        ni, d = g.shape[0], g.shape[-1]
        for _ in range(ni):
            fw_n = fwd[:, off : off + d * d].reshape(n, d, d)         # [t, p, r]
            kh_n = kh[:, :, off : off + d * d].reshape(F, C, d, d)    # [f, c, r, q]
            iv_n = inv[off : off + d * d, :].reshape(d, d, n)         # [p, q, s]
            W += np.einsum("tpr,fcrq,pqs->ctfs", fw_n, kh_n, iv_n, optimize=True)
            off += d * d
    Wflat = np.ascontiguousarray(W.reshape(CT, FS)).astype(np.float32)
    bias_fs = np.repeat(np.asarray(bias).astype(np.float32), NS)  # [FS], f-major
    # partition-major layout for the device: bias_pm[p, m] = bias_fs[m*128+p]
    bias_pm = np.ascontiguousarray(bias_fs.reshape(MT, 128).T)
    return Wflat, bias_pm


def build_kernel(nc: bass.Bass, bs: int = BS, reps: int = 1):
    """Emit the per-core kernel into `nc`. bs = batch rows for this build.

    reps > 1 wraps the whole pipeline in a hardware loop (for timing)."""
    assert bs % SUPER == 0
    import contextlib
    x_d = nc.dram_tensor("x", [bs, CT], MM_DT, kind="ExternalInput")
    w_d = nc.dram_tensor("w", [CT, FS], W_DT, kind="ExternalInput")
    b_d = nc.dram_tensor("bias_pm", [128, MT], mybir.dt.float32, kind="ExternalInput")
    id_d = nc.dram_tensor("ident", [128, 128], MM_DT, kind="ExternalInput")
    y_d = nc.dram_tensor("y", [bs, FS], mybir.dt.float32, kind="ExternalOutput")

    n_super = bs // SUPER
    KSUB = SUPER // 128  # 4 b-sub-blocks per super

    with TileContext(nc) as tc:
        with (
            tc.tile_pool(name="singles", bufs=1) as singles,
            tc.tile_pool(name="xin", bufs=6) as xin_pool,
            tc.tile_pool(name="xt", bufs=1) as xt_pool,
            tc.tile_pool(name="yt", bufs=1) as yt_pool,
            tc.tile_pool(name="ynat", bufs=3) as ynat_pool,
            tc.tile_pool(name="px", bufs=3, space="PSUM") as px_pool,
            tc.tile_pool(name="py", bufs=2, space="PSUM") as py_pool,
            tc.tile_pool(name="pyn", bufs=2, space="PSUM") as pyn_pool,
        ):
            ident = singles.tile([128, 128], MM_DT)
            nc.sync.dma_start(out=ident, in_=id_d[:, :])

            w_sb = singles.tile([128, KT, FS], W_DT)
            for j in range(KT):
                nc.sync.dma_start(
                    out=w_sb[:, j, :], in_=w_d[j * 128 : (j + 1) * 128, :]
                )
            bias_sb = singles.tile([128, MT], mybir.dt.float32)
            nc.sync.dma_start(out=bias_sb, in_=b_d[:, :])

            rep_ctx = (
                tc.For_i(0, reps, 1, hint_engines=(mybir.EngineType.PE,))
                if reps > 1
                else contextlib.nullcontext()
            )
            with rep_ctx:
                for sb in range(n_super):
                    r0 = sb * SUPER
                    # ---- load x natural, 4 blocks of [128, 1536]
                    x_nat = []
                    for k in range(KSUB):
                        xt_in = xin_pool.tile([128, CT], MM_DT, tag="xin")
                        nc.sync.dma_start(
                            out=xt_in,
                            in_=x_d[r0 + k * 128 : r0 + (k + 1) * 128, :],
                        )
                        x_nat.append(xt_in)

                    # ---- transpose x -> xT [128(ct-tile j), 512(b)]
                    xT = xt_pool.tile([128, KT, SUPER], W_DT, tag="xt")
                    for j in range(KT):
                        px = px_pool.tile([128, SUPER], MM_DT, tag="px")
                        for k in range(KSUB):
                            nc.tensor.transpose(
                                px[:, k * 128 : (k + 1) * 128],
                                x_nat[k][:, j * 128 : (j + 1) * 128],
                                ident,
                            )
                        # copybacks on ACT (DVE handles the y-side)
                        nc.scalar.copy(xT[:, j, :], px)

                    # ---- y^T[m] = sum_j W[j,m]^T @ xT[j]  (+bias on copyback)
                    yT = yt_pool.tile([128, MT, SUPER], MM_DT, tag="yt")
                    for m in range(MT):
                        py = py_pool.tile([128, SUPER], mybir.dt.float32, tag="py")
                        for j in range(KT):
                            nc.tensor.matmul(
                                py,
                                w_sb[:, j, m * 128 : (m + 1) * 128],
                                xT[:, j, :],
                                start=(j == 0),
                                stop=(j == KT - 1),
                            )
                        nc.vector.tensor_scalar_add(
                            yT[:, m, :], py, bias_sb[:, m : m + 1]
                        )

                    # ---- transpose back to natural y and store
                    for k in range(KSUB):
                        y_nat = ynat_pool.tile([128, FS], mybir.dt.float32, tag="ynat")
                        for mh in range(MT // 4):  # 4 m-tiles per PSUM bank
                            pyn = pyn_pool.tile([128, 512], MM_DT, tag="pyn")
                            for mm in range(4):
                                m = mh * 4 + mm
                                nc.tensor.transpose(
                                    pyn[:, mm * 128 : (mm + 1) * 128],
                                    yT[:, m, k * 128 : (k + 1) * 128],
                                    ident,
                                )
                            if mh % 2 == 0:
                                nc.vector.tensor_copy(
                                    y_nat[:, mh * 512 : (mh + 1) * 512], pyn
                                )
                            else:
                                nc.scalar.copy(
                                    y_nat[:, mh * 512 : (mh + 1) * 512], pyn
                                )
                        nc.sync.dma_start(
                            out=y_d[r0 + k * 128 : r0 + (k + 1) * 128, :], in_=y_nat
                        )
    return nc


def _run(x, Wflat, bias_pm, trace=False):
    nc = bacc.Bacc("TRN2", target_bir_lowering=False)
    build_kernel(nc, BS)
    nc.compile()
    xf = np.ascontiguousarray(x.reshape(B, CT))
    in_maps = [
        {
            "x": xf[i * BS : (i + 1) * BS],
            "w": Wflat.astype(np.float16),
            "bias_pm": bias_pm,
            "ident": np.eye(128, dtype=np.float32),
        }
        for i in range(N_CORES)
    ]
    res = run_bass_kernel_spmd(nc, in_maps, list(range(N_CORES)), trace=trace)
    y = np.concatenate([res.results[i]["y"] for i in range(N_CORES)], axis=0)
    return y.reshape(B, F, NS), res


def kernel(x, kernel, bias, irreps_d1, irreps_d2, irreps_d3):
    Wflat, bias_pm = _host_fold(kernel, bias, irreps_d1, irreps_d2, irreps_d3)
    y, _ = _run(np.asarray(x, dtype=np.float32), Wflat, bias_pm)
    return y

